# revision 1
# baseline (speedup 1.0000x reference)
"""
2-layer GAT on Trainium2 (8 NeuronCores, SPMD via bass/Tile).

Sharding: destination nodes are block-sharded across the 8 cores (6250
nodes each).  All per-edge work runs on the core owning the edge's dst.
Layer-0 node-level compute (h = x @ W1pack) is replicated on every core
(cheap), avoiding collectives for layer 0.  Layer 1 needs layer-0
output of all nodes, so the pipeline is two bass kernels with a host
gather of per-core node tables in between.

All per-edge feature movement uses dma_gather (InstDMAGatherAnt):
  - table1 [N, 256] bf16 rows = [h(128) | d1(8) | s1(8) | pad]  (512 B)
    gathered by src.  int16 idx limit (32767) is handled by splitting
    every dst tile's chunks into "lo" (src < SPLIT) and "hi" chunks,
    gathered from base row 0 / row SPLIT respectively.
  - table_s [N, 128] bf16 rows = [s1(8) | pad] (256 B) gathered by dst
    via the pair-row view [N/2, 256] with idx = dst>>1 (fits int16),
    then an even/odd select picks cols 0:8 vs 128:136.
  - kernel B: table2 [N, 128] bf16 = [feat2(16) | d2 | s2 | pad],
    pair-row gathers by src and by dst + parity selects.

Aggregation per dst tile of 128 nodes: for each chunk of 128 edges an
S one-hot (S[e,j] = dstlocal[e]==j, built on DVE by iota compare) and
a PE matmul accumulate psum[dst,:] += S.T @ [feat*ex | ex]; then
normalize by the summed ex.  Softmax max-subtraction is skipped: the
attention logits here are O(0.3) so exp() is stable, and softmax is
shift-invariant.
"""

import os
import sys
from contextlib import ExitStack

import numpy as np
import ml_dtypes

for _p in ("/opt/trn_rl_repo",):
    if os.path.isdir(_p) and _p not in sys.path:
        sys.path.insert(0, _p)

import concourse.bass as bass
import concourse.bacc as bacc
import concourse.tile as tile
from concourse import mybir
from concourse import bass_utils
from concourse._compat import with_exitstack

F32 = mybir.dt.float32
BF16 = mybir.dt.bfloat16
I32 = mybir.dt.int32
I16 = mybir.dt.int16
AF = mybir.ActivationFunctionType
OP = mybir.AluOpType
P = 128
BF = ml_dtypes.bfloat16


class Cfg:
    def __init__(self, N, E, ncores, split=32768, neg=0.2, in_ch=128,
                 f=128, heads=8, hid=16, out=16):
        self.N = N
        self.E = E
        self.NCORES = ncores
        self.SPLIT = split
        self.NEG = neg
        self.IN = in_ch
        self.F = f
        self.H = heads
        self.HID = hid
        self.OUT = out
        assert N % ncores == 0
        self.NPC = N // ncores
        self.TPC = (self.NPC + P - 1) // P
        self.NPC_PAD = self.TPC * P
        self.NTILES = ncores * self.TPC
        self.N_PAD = self.NTILES * P
        self.NCHL_T = None
        self.NCHH_T = None
        self.NCH_T = None
        self.NCH = None


def _wrap16(vals):
    """[n] slot-ordered values -> [128, n//16] int16 wrapped layout."""
    n = vals.shape[0]
    assert n % 16 == 0
    w = vals.reshape(-1, 16).T.astype(np.int16)      # [16, n//16]
    return np.ascontiguousarray(np.tile(w, (8, 1)))  # [128, n//16]


def _prep_graph(cfg, edge_index):
    N, NPC, SPL = cfg.N, cfg.NPC, cfg.SPLIT
    src = np.concatenate([edge_index[0], np.arange(N, dtype=np.int64)])
    dst = np.concatenate([edge_index[1], np.arange(N, dtype=np.int64)])
    core = dst // NPC
    ld = dst - core * NPC
    tile_id = ld // P
    dstloc = ld % P
    hi = (src >= SPL).astype(np.int64)
    order = np.lexsort((src, hi, tile_id, core))
    src, dst, core, tile_id, dstloc, hi = (a[order] for a in
                                           (src, dst, core, tile_id,
                                            dstloc, hi))
    keyf = (core * cfg.TPC + tile_id) * 2 + hi
    cntf = np.bincount(keyf, minlength=cfg.NCORES * cfg.TPC * 2)
    cnt_lo = cntf[0::2].reshape(cfg.NCORES, cfg.TPC)
    cnt_hi = cntf[1::2].reshape(cfg.NCORES, cfg.TPC)
    cfg.NCHL_T = max(1, int(np.max((cnt_lo + P - 1) // P)))
    cfg.NCHH_T = max(1, int(np.max((cnt_hi + P - 1) // P)))
    cfg.NCH_T = cfg.NCHL_T + cfg.NCHH_T
    cfg.NCH = cfg.NCH_T * cfg.TPC
    starts = np.concatenate([[0], np.cumsum(cntf)])

    pc = dict(srcw_lo=[], srcw_hi=[], dstw=[], srcw=[], dstloc_f=[],
              dst_par=[], src_par=[])
    for c in range(cfg.NCORES):
        ns_lo = cfg.TPC * cfg.NCHL_T * P
        ns_hi = cfg.TPC * cfg.NCHH_T * P
        ns = cfg.NCH * P
        v_srclo = np.zeros(ns_lo, np.int64)
        v_srchi = np.zeros(ns_hi, np.int64)
        v_dst = np.zeros(ns, np.int64)          # dst>>1 per slot
        v_src = np.zeros(ns, np.int64)          # src>>1 per slot
        v_dstloc = np.full(ns, -1.0, np.float32)
        v_dpar = np.ones(ns, np.float32)
        v_spar = np.ones(ns, np.float32)
        for t in range(cfg.TPC):
            for h in (0, 1):
                k = ((c * cfg.TPC + t) * 2 + h)
                n = int(cntf[k])
                if n == 0:
                    continue
                sl = slice(starts[k], starts[k] + n)
                e_src = src[sl]
                e_dst = dst[sl]
                e_dl = dstloc[sl]
                pos = np.arange(n)
                if h == 0:
                    v_srclo[t * cfg.NCHL_T * P + pos] = e_src
                    ch = t * cfg.NCH_T + pos // P
                else:
                    v_srchi[t * cfg.NCHH_T * P + pos] = e_src - SPL
                    ch = t * cfg.NCH_T + cfg.NCHL_T + pos // P
                slot = ch * P + pos % P
                v_dst[slot] = e_dst >> 1
                v_src[slot] = e_src >> 1
                v_dstloc[slot] = e_dl
                v_dpar[slot] = 1.0 - (e_dst & 1)
                v_spar[slot] = 1.0 - (e_src & 1)
        pc["srcw_lo"].append(_wrap16(v_srclo))
        pc["srcw_hi"].append(_wrap16(v_srchi))
        pc["dstw"].append(_wrap16(v_dst))
        pc["srcw"].append(_wrap16(v_src))
        # slot arrays in [128, NCH] layout: slot = ch*128 + p -> [p, ch]
        pc["dstloc_f"].append(
            np.ascontiguousarray(v_dstloc.reshape(cfg.NCH, P).T))
        pc["dst_par"].append(np.ascontiguousarray(
            v_dpar.reshape(cfg.NCH, P).T))
        pc["src_par"].append(np.ascontiguousarray(
            v_spar.reshape(cfg.NCH, P).T))
    return pc


def _blockdiag_att(att, heads, hid, f):
    A = np.zeros((f, heads), dtype=np.float32)
    for h in range(heads):
        A[h * hid:(h + 1) * hid, h] = att[0, h]
    return A


def _ap(base, ap_list, off_extra=0):
    return bass.AP(tensor=base.tensor, offset=base.offset + off_extra,
                   ap=ap_list)


@with_exitstack
def _build_a(ctx, tc, cfg, t, repeat=1):
    for _rep in range(repeat):
        _build_a_once(tc, cfg, t)
        if _rep < repeat - 1:
            tc.strict_bb_all_engine_barrier()


@with_exitstack
def _build_a_once(ctx, tc, cfg, t):
    nc = tc.nc
    NCH_T, NCHL_T, NCHH_T, TPC = cfg.NCH_T, cfg.NCHL_T, cfg.NCHH_T, cfg.TPC
    WCOLS = cfg.F + 2 * cfg.H             # 144 matmul out cols
    TCOLS = cfg.F + cfg.H                 # 136 table1 used cols + s sep
    MCOLS = cfg.F + cfg.H                 # 136 message cols
    ROW1 = 256                            # table1 row elems (bf16)
    ROWS = 128                            # table_s row elems (bf16)
    W2COLS = cfg.OUT + 2

    consts = ctx.enter_context(tc.tile_pool(name="consts", bufs=1))
    wpack = consts.tile([P, WCOLS], F32)
    nc.sync.dma_start(out=wpack[:], in_=t["wpack"][:, :])
    w2pack = consts.tile([P, W2COLS], F32)
    nc.sync.dma_start(out=w2pack[:], in_=t["w2pack"][:, :])
    iota = consts.tile([P, P], F32)
    nc.sync.dma_start(out=iota[:], in_=t["iota"][:, :])
    ident = consts.tile([P, P], F32)
    nc.sync.dma_start(out=ident[:], in_=t["ident"][:, :])

    # ---------------- node phase ----------------
    NT = cfg.NTILES
    BLK = 8
    with ExitStack() as nctx:
        xpool = nctx.enter_context(tc.tile_pool(name="xt", bufs=2))
        npsum = nctx.enter_context(tc.tile_pool(name="npsum", bufs=2,
                                                space="PSUM"))
        nstage = nctx.enter_context(tc.tile_pool(name="nstage", bufs=3))
        for blk in range((NT + BLK - 1) // BLK):
            nt0 = blk * BLK
            nt1 = min(nt0 + BLK, NT)
            xt = xpool.tile([P, BLK * P], F32, tag="xt")
            nc.sync.dma_start(out=xt[:, 0:(nt1 - nt0) * P],
                              in_=t["xT"][:, nt0 * P:nt1 * P])
            for j in range(nt1 - nt0):
                nt = nt0 + j
                pt = npsum.tile([P, WCOLS], F32, tag="npt")
                nc.tensor.matmul(out=pt[:], lhsT=xt[:, j * P:(j + 1) * P],
                                 rhs=wpack[:], start=True, stop=True)
                s1 = nstage.tile([P, TCOLS], BF16, tag="s1")
                nc.scalar.activation(s1[:], pt[:, 0:TCOLS], AF.Copy)
                ss = nstage.tile([P, cfg.H], BF16, tag="ss")
                nc.scalar.activation(ss[:], pt[:, TCOLS:WCOLS], AF.Copy)
                nc.sync.dma_start(
                    out=t["table1"][nt * P:(nt + 1) * P, 0:TCOLS], in_=s1[:])
                nc.sync.dma_start(
                    out=t["table_s"][nt * P:(nt + 1) * P, 0:cfg.H],
                    in_=ss[:])

    # Tile does not track DRAM deps: fence table writes vs gathers.
    tc.strict_bb_all_engine_barrier()

    # ---------------- edge phase ----------------
    gpool = ctx.enter_context(tc.tile_pool(name="g", bufs=2))
    gspool = ctx.enter_context(tc.tile_pool(name="gs", bufs=2))
    ipool = ctx.enter_context(tc.tile_pool(name="idx", bufs=2))
    spool = ctx.enter_context(tc.tile_pool(name="sel", bufs=2))
    mpool = ctx.enter_context(tc.tile_pool(name="msg", bufs=2))
    lpool = ctx.enter_context(tc.tile_pool(name="logit", bufs=2))
    apsum = ctx.enter_context(tc.tile_pool(name="apsum", bufs=2,
                                           space="PSUM"))
    tpsum = ctx.enter_context(tc.tile_pool(name="tpsum", bufs=2,
                                           space="PSUM"))
    t2psum = ctx.enter_context(tc.tile_pool(name="t2psum", bufs=2,
                                            space="PSUM"))
    hpool = ctx.enter_context(tc.tile_pool(name="h1", bufs=2))

    tabs_pair = _ap(t["table_s"][:, :], [[256, cfg.N_PAD // 2], [1, 256]])
    tab_hi = t["table1"][cfg.SPLIT:cfg.N_PAD, :]

    for ti in range(TPC):
        c0 = ti * NCH_T
        # per-tile index tiles
        il = ipool.tile([P, NCHL_T * 8], I16, tag="il")
        nc.sync.dma_start(out=il[:], in_=t["srcw_lo"][
            :, ti * NCHL_T * 8:(ti + 1) * NCHL_T * 8])
        ih = ipool.tile([P, NCHH_T * 8], I16, tag="ih")
        nc.sync.dma_start(out=ih[:], in_=t["srcw_hi"][
            :, ti * NCHH_T * 8:(ti + 1) * NCHH_T * 8])
        idst = ipool.tile([P, NCH_T * 8], I16, tag="idst")
        nc.sync.dma_start(out=idst[:], in_=t["dstw"][
            :, ti * NCH_T * 8:(ti + 1) * NCH_T * 8])
        dloc = ipool.tile([P, NCH_T], F32, tag="dloc")
        nc.sync.dma_start(out=dloc[:], in_=t["dstloc_f"][:, c0:c0 + NCH_T])
        dpar = ipool.tile([P, NCH_T], F32, tag="dpar")
        nc.sync.dma_start(out=dpar[:], in_=t["dst_par"][:, c0:c0 + NCH_T])

        # gathers
        G = gpool.tile([P, NCH_T, ROW1], BF16, tag="G")
        nc.gpsimd.dma_gather(
            out_ap=G[:, 0:NCHL_T, :], in_ap=t["table1"][:, :],
            idxs_ap=il[:], num_idxs=NCHL_T * P, num_idxs_reg=NCHL_T * P,
            elem_size=ROW1, single_packet=False)
        nc.gpsimd.dma_gather(
            out_ap=G[:, NCHL_T:NCH_T, :], in_ap=tab_hi,
            idxs_ap=ih[:], num_idxs=NCHH_T * P, num_idxs_reg=NCHH_T * P,
            elem_size=ROW1, single_packet=False)
        Gs = gspool.tile([P, NCH_T, 256], BF16, tag="Gs")
        nc.gpsimd.dma_gather(
            out_ap=Gs[:], in_ap=tabs_pair, idxs_ap=idst[:],
            num_idxs=NCH_T * P, num_idxs_reg=NCH_T * P, elem_size=256, single_packet=False)

        # s1[dst] via parity select: s = odd + par*(even - odd)
        sd = spool.tile([P, NCH_T, cfg.H], F32, tag="sd")
        nc.vector.tensor_tensor(out=sd[:], in0=Gs[:, :, 0:cfg.H],
                                in1=Gs[:, :, ROWS:ROWS + cfg.H],
                                op=OP.subtract)
        sm = spool.tile([P, NCH_T, cfg.H], F32, tag="sm")
        nc.vector.tensor_tensor(
            out=sm[:], in0=sd[:],
            in1=_ap(dpar[:], [dpar[:].ap[0], [1, NCH_T], [0, cfg.H]]),
            op=OP.mult)
        so = spool.tile([P, NCH_T, cfg.H], F32, tag="so")
        nc.scalar.activation(so[:], Gs[:, :, ROWS:ROWS + cfg.H], AF.Copy)
        sp = spool.tile([P, NCH_T, cfg.H], F32, tag="sp")
        nc.vector.tensor_tensor(out=sp[:], in0=sm[:], in1=so[:], op=OP.add)

        # one-hot S
        S = spool.tile([P, NCH_T, P], BF16, tag="S")
        nc.vector.tensor_tensor(
            out=S[:],
            in0=_ap(iota[:], [iota[:].ap[0], [0, NCH_T], [1, P]]),
            in1=_ap(dloc[:], [dloc[:].ap[0], [1, NCH_T], [0, P]]),
            op=OP.is_equal)

        # logits -> ex (bf16)
        dcp = lpool.tile([P, NCH_T, cfg.H], F32, tag="dcp")
        nc.scalar.activation(dcp[:], G[:, :, cfg.F:cfg.F + cfg.H], AF.Copy)
        u = lpool.tile([P, NCH_T, cfg.H], F32, tag="u")
        nc.vector.tensor_tensor(out=u[:], in0=sp[:], in1=dcp[:], op=OP.add)
        a = lpool.tile([P, NCH_T, cfg.H], F32, tag="a")
        nc.vector.scalar_tensor_tensor(out=a[:], in0=u[:], scalar=cfg.NEG,
                                       in1=u[:], op0=OP.mult, op1=OP.max)
        ex = lpool.tile([P, NCH_T, cfg.H], BF16, tag="ex")
        nc.scalar.activation(ex[:], a[:], AF.Exp)

        # Msg = [feat * ex | ex]
        M = mpool.tile([P, NCH_T, MCOLS], BF16, tag="M")
        nc.scalar.activation(M[:, :, cfg.F:MCOLS], ex[:], AF.Copy)
        nc.vector.tensor_tensor(
            out=_ap(M[:], [M[:].ap[0], [MCOLS, NCH_T], [cfg.HID, cfg.H],
                           [1, cfg.HID]]),
            in0=_ap(G[:], [G[:].ap[0], [ROW1, NCH_T], [cfg.HID, cfg.H],
                           [1, cfg.HID]]),
            in1=_ap(ex[:], [ex[:].ap[0], [cfg.H, NCH_T], [1, cfg.H],
                            [0, cfg.HID]]),
            op=OP.mult)

        # aggregate
        agg = apsum.tile([P, MCOLS], F32, tag="agg")
        for k in range(NCH_T):
            nc.tensor.matmul(out=agg[:], lhsT=S[:, k, :], rhs=M[:, k, :],
                             start=(k == 0), stop=(k == NCH_T - 1))

        # normalize + elu + feat2/d2/s2 slab
        den = hpool.tile([P, cfg.H], F32, tag="den")
        nc.vector.tensor_scalar_add(den[:], agg[:, cfg.F:MCOLS], 1e-20)
        rcp = hpool.tile([P, cfg.H], F32, tag="rcp")
        nc.vector.reciprocal(rcp[:], den[:])
        h1 = hpool.tile([P, cfg.F], F32, tag="h1")
        nc.vector.tensor_tensor(
            out=_ap(h1[:], [h1[:].ap[0], [cfg.HID, cfg.H], [1, cfg.HID]]),
            in0=_ap(agg[:], [agg[:].ap[0], [cfg.HID, cfg.H], [1, cfg.HID]]),
            in1=_ap(rcp[:], [rcp[:].ap[0], [1, cfg.H], [0, cfg.HID]]),
            op=OP.mult)
        neg = hpool.tile([P, cfg.F], F32, tag="neg")
        nc.vector.tensor_scalar_min(neg[:], h1[:], 0.0)
        pos = hpool.tile([P, cfg.F], F32, tag="pos")
        nc.vector.tensor_scalar_max(pos[:], h1[:], 0.0)
        een = hpool.tile([P, cfg.F], F32, tag="een")
        nc.scalar.activation(een[:], neg[:], AF.Exp)
        elu = hpool.tile([P, cfg.F], F32, tag="elu")
        nc.vector.scalar_tensor_tensor(out=elu[:], in0=een[:], scalar=-1.0,
                                       in1=pos[:], op0=OP.add, op1=OP.add)
        eT_ps = tpsum.tile([P, P], F32, tag="eT")
        nc.tensor.transpose(out=eT_ps[:], in_=elu[:], identity=ident[:])
        eT = hpool.tile([P, P], F32, tag="eTs")
        nc.scalar.activation(eT[:], eT_ps[:], AF.Copy)
        t2 = t2psum.tile([P, W2COLS], F32, tag="t2")
        nc.tensor.matmul(out=t2[:], lhsT=eT[:], rhs=w2pack[:],
                         start=True, stop=True)
        t2s = hpool.tile([P, W2COLS], F32, tag="t2s")
        nc.scalar.activation(t2s[:], t2[:], AF.Copy)
        nc.sync.dma_start(out=t["table2slab"][ti * P:(ti + 1) * P, :],
                          in_=t2s[:])


@with_exitstack
def _build_b(ctx, tc, cfg, t, repeat=1):
    for _rep in range(repeat):
        _build_b_once(tc, cfg, t)
        if _rep < repeat - 1:
            tc.strict_bb_all_engine_barrier()


@with_exitstack
def _build_b_once(ctx, tc, cfg, t):
    nc = tc.nc
    NCH_T, TPC = cfg.NCH_T, cfg.TPC
    UC = cfg.OUT + 2                    # 18 used cols in table2
    MC = cfg.OUT + 1                    # 17 message cols

    consts = ctx.enter_context(tc.tile_pool(name="consts", bufs=1))
    iota = consts.tile([P, P], F32)
    nc.sync.dma_start(out=iota[:], in_=t["iota"][:, :])

    ipool = ctx.enter_context(tc.tile_pool(name="idx", bufs=2))
    gpool = ctx.enter_context(tc.tile_pool(name="g2", bufs=2))
    spool = ctx.enter_context(tc.tile_pool(name="s2", bufs=2))
    lpool = ctx.enter_context(tc.tile_pool(name="l2", bufs=2))
    mpool = ctx.enter_context(tc.tile_pool(name="m2", bufs=2))
    apsum = ctx.enter_context(tc.tile_pool(name="aps2", bufs=2,
                                           space="PSUM"))
    opool = ctx.enter_context(tc.tile_pool(name="o", bufs=3))

    tab_pair = _ap(t["table2"][:, :], [[256, cfg.N_PAD // 2], [1, 256]])

    for ti in range(TPC):
        c0 = ti * NCH_T
        isrc = ipool.tile([P, NCH_T * 8], I16, tag="isrc")
        nc.sync.dma_start(out=isrc[:], in_=t["srcw"][
            :, ti * NCH_T * 8:(ti + 1) * NCH_T * 8])
        idst = ipool.tile([P, NCH_T * 8], I16, tag="idst")
        nc.sync.dma_start(out=idst[:], in_=t["dstw"][
            :, ti * NCH_T * 8:(ti + 1) * NCH_T * 8])
        dloc = ipool.tile([P, NCH_T], F32, tag="dloc")
        nc.sync.dma_start(out=dloc[:], in_=t["dstloc_f"][:, c0:c0 + NCH_T])
        dpar = ipool.tile([P, NCH_T], F32, tag="dpar")
        nc.sync.dma_start(out=dpar[:], in_=t["dst_par"][:, c0:c0 + NCH_T])
        spar = ipool.tile([P, NCH_T], F32, tag="spar")
        nc.sync.dma_start(out=spar[:], in_=t["src_par"][:, c0:c0 + NCH_T])

        G2 = gpool.tile([P, NCH_T, 256], BF16, tag="G2")
        nc.gpsimd.dma_gather(
            out_ap=G2[:], in_ap=tab_pair, idxs_ap=isrc[:],
            num_idxs=NCH_T * P, num_idxs_reg=NCH_T * P, elem_size=256, single_packet=False)
        Gd2 = gpool.tile([P, NCH_T, 256], BF16, tag="Gd2")
        nc.gpsimd.dma_gather(
            out_ap=Gd2[:], in_ap=tab_pair, idxs_ap=idst[:],
            num_idxs=NCH_T * P, num_idxs_reg=NCH_T * P, elem_size=256, single_packet=False)

        # parity selects: x = odd + par*(even - odd)
        Rd = spool.tile([P, NCH_T, UC], F32, tag="Rd")
        nc.vector.tensor_tensor(out=Rd[:], in0=G2[:, :, 0:UC],
                                in1=G2[:, :, 128:128 + UC], op=OP.subtract)
        Rm = spool.tile([P, NCH_T, UC], F32, tag="Rm")
        nc.vector.tensor_tensor(
            out=Rm[:], in0=Rd[:],
            in1=_ap(spar[:], [spar[:].ap[0], [1, NCH_T], [0, UC]]),
            op=OP.mult)
        Ro = spool.tile([P, NCH_T, UC], F32, tag="Ro")
        nc.scalar.activation(Ro[:], G2[:, :, 128:128 + UC], AF.Copy)
        R = spool.tile([P, NCH_T, UC], F32, tag="R")
        nc.vector.tensor_tensor(out=R[:], in0=Rm[:], in1=Ro[:], op=OP.add)

        s2d = spool.tile([P, NCH_T, 1], F32, tag="s2d")
        nc.vector.tensor_tensor(out=s2d[:], in0=Gd2[:, :, MC:MC + 1],
                                in1=Gd2[:, :, 128 + MC:128 + MC + 1],
                                op=OP.subtract)
        s2m = spool.tile([P, NCH_T, 1], F32, tag="s2m")
        nc.vector.tensor_tensor(
            out=s2m[:], in0=s2d[:],
            in1=_ap(dpar[:], [dpar[:].ap[0], [1, NCH_T], [0, 1]]),
            op=OP.mult)
        s2o = spool.tile([P, NCH_T, 1], F32, tag="s2o")
        nc.scalar.activation(s2o[:], Gd2[:, :, 128 + MC:128 + MC + 1],
                             AF.Copy)
        s2 = spool.tile([P, NCH_T, 1], F32, tag="s2")
        nc.vector.tensor_tensor(out=s2[:], in0=s2m[:], in1=s2o[:], op=OP.add)

        S = spool.tile([P, NCH_T, P], BF16, tag="S")
        nc.vector.tensor_tensor(
            out=S[:],
            in0=_ap(iota[:], [iota[:].ap[0], [0, NCH_T], [1, P]]),
            in1=_ap(dloc[:], [dloc[:].ap[0], [1, NCH_T], [0, P]]),
            op=OP.is_equal)

        u = lpool.tile([P, NCH_T, 1], F32, tag="u2")
        nc.vector.tensor_tensor(out=u[:], in0=s2[:],
                                in1=R[:, :, cfg.OUT:MC], op=OP.add)
        a = lpool.tile([P, NCH_T, 1], F32, tag="a2")
        nc.vector.scalar_tensor_tensor(out=a[:], in0=u[:], scalar=cfg.NEG,
                                       in1=u[:], op0=OP.mult, op1=OP.max)
        ex = lpool.tile([P, NCH_T, 1], F32, tag="ex2")
        nc.scalar.activation(ex[:], a[:], AF.Exp)

        M = mpool.tile([P, NCH_T, MC], BF16, tag="M2")
        nc.scalar.activation(M[:, :, cfg.OUT:MC], ex[:], AF.Copy)
        nc.vector.tensor_tensor(
            out=M[:, :, 0:cfg.OUT],
            in0=R[:, :, 0:cfg.OUT],
            in1=_ap(ex[:], [ex[:].ap[0], [1, NCH_T], [0, cfg.OUT]]),
            op=OP.mult)

        agg = apsum.tile([P, MC], F32, tag="agg2")
        for k in range(NCH_T):
            nc.tensor.matmul(out=agg[:], lhsT=S[:, k, :], rhs=M[:, k, :],
                             start=(k == 0), stop=(k == NCH_T - 1))

        den = opool.tile([P, 1], F32, tag="den")
        nc.vector.tensor_scalar_add(den[:], agg[:, cfg.OUT:MC], 1e-20)
        rcp = opool.tile([P, 1], F32, tag="rcp")
        nc.vector.reciprocal(rcp[:], den[:])
        h2 = opool.tile([P, cfg.OUT], F32, tag="h2")
        nc.vector.tensor_tensor(
            out=h2[:], in0=agg[:, 0:cfg.OUT],
            in1=_ap(rcp[:], [rcp[:].ap[0], [0, cfg.OUT]]), op=OP.mult)
        m = opool.tile([P, 1], F32, tag="m")
        nc.vector.tensor_reduce(out=m[:], in_=h2[:],
                                axis=mybir.AxisListType.X, op=OP.max)
        tm = opool.tile([P, cfg.OUT], F32, tag="tm")
        nc.vector.tensor_tensor(
            out=tm[:], in0=h2[:],
            in1=_ap(m[:], [m[:].ap[0], [0, cfg.OUT]]), op=OP.subtract)
        pe = opool.tile([P, cfg.OUT], F32, tag="pe")
        ssum = opool.tile([P, 1], F32, tag="ss")
        nc.scalar.activation(pe[:], tm[:], AF.Exp, accum_out=ssum[:])
        ln = opool.tile([P, 1], F32, tag="ln")
        nc.scalar.activation(ln[:], ssum[:], AF.Ln)
        res = opool.tile([P, cfg.OUT], F32, tag="res")
        nc.vector.tensor_tensor(
            out=res[:], in0=tm[:],
            in1=_ap(ln[:], [ln[:].ap[0], [0, cfg.OUT]]), op=OP.subtract)
        nc.sync.dma_start(out=t["outp"][ti * P:(ti + 1) * P, :], in_=res[:])


def _decl_a(nc, cfg):
    t = {}
    WCOLS = cfg.F + 2 * cfg.H
    W2COLS = cfg.OUT + 2

    def inp(name, shape, dt):
        t[name] = nc.dram_tensor(name, shape, dt, kind="ExternalInput").ap()

    inp("xT", [P, cfg.N_PAD], F32)
    inp("wpack", [P, WCOLS], F32)
    inp("w2pack", [P, W2COLS], F32)
    inp("iota", [P, P], F32)
    inp("ident", [P, P], F32)
    inp("srcw_lo", [P, cfg.TPC * cfg.NCHL_T * 8], I16)
    inp("srcw_hi", [P, cfg.TPC * cfg.NCHH_T * 8], I16)
    inp("dstw", [P, cfg.NCH * 8], I16)
    inp("dstloc_f", [P, cfg.NCH], F32)
    inp("dst_par", [P, cfg.NCH], F32)
    t["table1"] = nc.dram_tensor("table1", [cfg.N_PAD, 256], BF16,
                                 kind="Internal").ap()
    t["table_s"] = nc.dram_tensor("table_s", [cfg.N_PAD, 128], BF16,
                                  kind="Internal").ap()
    t["table2slab"] = nc.dram_tensor("table2slab", [cfg.NPC_PAD, W2COLS],
                                     F32, kind="ExternalOutput").ap()
    return t


def _decl_b(nc, cfg):
    t = {}

    def inp(name, shape, dt):
        t[name] = nc.dram_tensor(name, shape, dt, kind="ExternalInput").ap()

    inp("table2", [cfg.N_PAD, 128], BF16)
    inp("srcw", [P, cfg.NCH * 8], I16)
    inp("dstw", [P, cfg.NCH * 8], I16)
    inp("dstloc_f", [P, cfg.NCH], F32)
    inp("dst_par", [P, cfg.NCH], F32)
    inp("src_par", [P, cfg.NCH], F32)
    inp("iota", [P, P], F32)
    t["outp"] = nc.dram_tensor("outp", [cfg.NPC_PAD, cfg.OUT], F32,
                               kind="ExternalOutput").ap()
    return t


def _compile(build_fn, decl_fn, cfg, repeat=1):
    nc = bacc.Bacc("TRN2", target_bir_lowering=False, debug=False,
                   enable_asserts=False, num_devices=cfg.NCORES)
    t = decl_fn(nc, cfg)
    with tile.TileContext(nc) as tc:
        build_fn(tc, cfg, t, repeat=repeat)
    nc.compile()
    return nc


def _host_prep_weights(cfg, W1, att_src1, att_dst1, W2, att_src2, att_dst2):
    A_d1 = _blockdiag_att(np.asarray(att_dst1, np.float32), cfg.H, cfg.HID,
                          cfg.F)
    A_s1 = _blockdiag_att(np.asarray(att_src1, np.float32), cfg.H, cfg.HID,
                          cfg.F)
    W1T = np.asarray(W1, np.float32).T.copy()
    wpack = np.concatenate([W1T, W1T @ A_d1, W1T @ A_s1], axis=1)
    W2T = np.asarray(W2, np.float32).T.copy()
    a_d2 = np.asarray(att_dst2, np.float32).reshape(cfg.OUT, 1)
    a_s2 = np.asarray(att_src2, np.float32).reshape(cfg.OUT, 1)
    w2pack = np.concatenate([W2T, W2T @ a_d2, W2T @ a_s2], axis=1)
    return (np.ascontiguousarray(wpack, np.float32),
            np.ascontiguousarray(w2pack, np.float32))


_CACHE = {}


def _get_kernels(cfg):
    key = (cfg.N, cfg.E, cfg.NCORES, cfg.NCH_T, cfg.NCHL_T)
    if key not in _CACHE:
        nca = _compile(_build_a, _decl_a, cfg)
        ncb = _compile(_build_b, _decl_b, cfg)
        _CACHE[key] = (nca, ncb)
    return _CACHE[key]


def run(cfg, inputs, runner=None):
    x = np.asarray(inputs["x"], np.float32)
    edge_index = np.asarray(inputs["edge_index"], np.int64)
    pc = _prep_graph(cfg, edge_index)
    wpack, w2pack = _host_prep_weights(
        cfg, inputs["W1"], inputs["att_src1"], inputs["att_dst1"],
        inputs["W2"], inputs["att_src2"], inputs["att_dst2"])

    xT = np.zeros((P, cfg.N_PAD), np.float32)
    xT[:, :cfg.N] = x.T
    iota = np.tile(np.arange(P, dtype=np.float32), (P, 1))
    ident = np.eye(P, dtype=np.float32)

    nca, ncb = _get_kernels(cfg)

    if runner is None:
        def runner(nc, in_maps):
            r = bass_utils.run_bass_kernel_spmd(
                nc, in_maps, core_ids=list(range(cfg.NCORES)))
            return r.results

    in_maps_a = []
    for c in range(cfg.NCORES):
        in_maps_a.append(dict(
            xT=xT, wpack=wpack, w2pack=w2pack, iota=iota, ident=ident,
            srcw_lo=pc["srcw_lo"][c], srcw_hi=pc["srcw_hi"][c],
            dstw=pc["dstw"][c], dstloc_f=pc["dstloc_f"][c],
            dst_par=pc["dst_par"][c]))
    res_a = runner(nca, in_maps_a)

    table2 = np.zeros((cfg.N_PAD, 128), BF)
    for c in range(cfg.NCORES):
        slab = np.asarray(res_a[c]["table2slab"], np.float32)
        table2[c * cfg.NPC:(c + 1) * cfg.NPC, 0:cfg.OUT + 2] = \
            slab[:cfg.NPC].astype(BF)

    in_maps_b = []
    for c in range(cfg.NCORES):
        in_maps_b.append(dict(
            table2=table2, srcw=pc["srcw"][c], dstw=pc["dstw"][c],
            dstloc_f=pc["dstloc_f"][c], dst_par=pc["dst_par"][c],
            src_par=pc["src_par"][c], iota=iota))
    res_b = runner(ncb, in_maps_b)

    out = np.zeros((cfg.N, cfg.OUT), np.float32)
    for c in range(cfg.NCORES):
        out[c * cfg.NPC:(c + 1) * cfg.NPC] = \
            np.asarray(res_b[c]["outp"], np.float32)[:cfg.NPC]
    return out


def kernel(**inputs):
    cfg = Cfg(N=50000, E=1600000, ncores=8)
    return run(cfg, inputs)



# revision 5
# speedup vs baseline: 1.5401x; 1.5401x over previous
"""
2-layer GAT on Trainium2 (8 NeuronCores, SPMD via bass/Tile) — v2.

Design notes (what makes this fast):

The dominant cost on TRN2 for random-edge GNN aggregation is SWDGE
descriptor generation for dma_gather: ~8ns per gathered element on one
queue, measured.  So the kernel is built around exactly ONE gather
element per edge per layer, generated on 4 parallel SWDGE queues
(~2ns/elem):

- Nodes are permuted (host-side, from edge_index only): each core's dst
  nodes are sorted by in-degree (desc) and tiled 128 at a time.  Within
  a tile every dst sits at a fixed partition; edge slot (p, k) holds the
  k-th in-edge of the p-th dst.  Degree sorting makes per-tile max
  degree ~= mean degree, so slot padding is only a few %.
- Because slot partition == dst, aggregation is a plain free-axis
  reduce (DVE), the dst-side attention term is partition-aligned (no
  per-edge gather/select for it), and no one-hot matrices exist at all.
- The int16 gather-index limit (N=50176 > 32767) is dodged by gathering
  PAIRS of table rows (idx = row>>1, 1024B element) and resolving row
  parity on DVE with host-streamed (1-par)/par weights, fused into the
  message-scaling multiply.
- Layer tables are written in a per-core ROTATED row order (own nodes
  first) so the same SPMD program can stash the dst-side attention
  columns for its own tiles at compile-time offsets.
- Between the two layer kernels the host only repacks device-computed
  numbers (layer-2 node table) — all FLOPs happen on device.
"""

import os
import sys
from contextlib import ExitStack

import numpy as np
import ml_dtypes

for _p in ("/opt/trn_rl_repo",):
    if os.path.isdir(_p) and _p not in sys.path:
        sys.path.insert(0, _p)

import concourse.bass as bass
import concourse.bacc as bacc
import concourse.tile as tile
from concourse import mybir
from concourse import bass_utils
from concourse._compat import with_exitstack

F32 = mybir.dt.float32
BF16 = mybir.dt.bfloat16
I16 = mybir.dt.int16
AF = mybir.ActivationFunctionType
OP = mybir.AluOpType
P = 128
BF = ml_dtypes.bfloat16
NQ = 4          # SWDGE queues


class Cfg:
    def __init__(self, N, E, ncores, neg=0.2, in_ch=128, f=128, heads=8,
                 hid=16, out=16):
        self.N = N
        self.E = E
        self.NCORES = ncores
        self.NEG = neg
        self.IN = in_ch
        self.F = f
        self.H = heads
        self.HID = hid
        self.OUT = out
        self.NPC = N // ncores                    # 6250
        self.TPC = (self.NPC + P - 1) // P        # 49
        self.NPC_PAD = self.TPC * P               # 6272
        self.N_PAD = ncores * self.NPC_PAD        # 50176
        self.NTILES = ncores * self.TPC           # 392
        self.NCH = None       # [TPC] chunks per tile (shared across cores)
        self.NCHSUM = None    # sum(NCH)


def _wrap16(vals):
    """[n] slot-ordered int values -> [128, n//16] int16 wrapped layout."""
    n = vals.shape[0]
    assert n % 16 == 0
    w = vals.reshape(-1, 16).T.astype(np.int16)
    return np.ascontiguousarray(np.tile(w, (8, 1)))


def _prep_graph(cfg, edge_index):
    """Degree-sorted slot layout. Returns per-core arrays + permutation."""
    N, NPC, NPC_PAD, TPC, C = cfg.N, cfg.NPC, cfg.NPC_PAD, cfg.TPC, cfg.NCORES
    src = np.concatenate([edge_index[0], np.arange(N, dtype=np.int64)])
    dst = np.concatenate([edge_index[1], np.arange(N, dtype=np.int64)])

    deg = np.bincount(dst, minlength=N)
    core = dst // NPC

    # permutation: within each core, nodes sorted by degree desc
    pi = np.empty(N, np.int64)           # old node -> pi row
    for c in range(C):
        nodes = np.arange(c * NPC, (c + 1) * NPC)
        order = nodes[np.argsort(-deg[nodes], kind="stable")]
        pi[order] = c * NPC_PAD + np.arange(NPC)

    rank = pi[dst] - core * NPC_PAD      # 0..NPC-1 within core
    tile_id = rank // P
    p_part = rank % P

    # edge rank k within its dst (order of appearance)
    order = np.lexsort((src, dst))
    k_sorted = np.arange(len(dst)) - np.repeat(
        np.concatenate([[0], np.cumsum(np.bincount(dst, minlength=N))[:-1]]),
        np.bincount(dst, minlength=N))
    k = np.empty(len(dst), np.int64)
    k[order] = k_sorted

    # chunks per tile: max over cores of per-(core,tile) max degree
    mx = np.zeros(C * TPC, np.int64)
    np.maximum.at(mx, core * TPC + tile_id, k + 1)
    NCH = np.maximum(mx.reshape(C, TPC).max(axis=0), 1)
    cfg.NCH = [int(x) for x in NCH]
    cfg.NCHSUM = int(NCH.sum())
    tile_base = np.concatenate([[0], np.cumsum(NCH)])   # in chunks

    pis = pi[src]
    pc = dict(idxA=[], idxB=[], pe=[], po=[], mq4=[])
    for c in range(C):
        m = core == c
        rot = (pis[m] - c * NPC_PAD) % cfg.N_PAD
        slot = (tile_base[tile_id[m]] + k[m]) * P + p_part[m]
        ns = cfg.NCHSUM * P
        vA = np.zeros(ns, np.int64)
        vB = np.zeros(ns, np.int64)
        vpe = np.zeros(ns, np.float32)
        vpo = np.zeros(ns, np.float32)
        vq = np.zeros((ns, 4), np.float32)
        vA[slot] = rot >> 1
        vB[slot] = rot >> 2
        par = rot & 1
        vpe[slot] = 1.0 - par
        vpo[slot] = par
        vq[slot, rot & 3] = 1.0
        pc["idxA"].append(_wrap16(vA))
        pc["idxB"].append(_wrap16(vB))
        # [128, NCHSUM] layouts (partition = slot p, col = chunk)
        pc["pe"].append(np.ascontiguousarray(
            vpe.reshape(cfg.NCHSUM, P).T.astype(BF)))
        pc["po"].append(np.ascontiguousarray(
            vpo.reshape(cfg.NCHSUM, P).T.astype(BF)))
        # [128, NCHSUM*4]
        pc["mq4"].append(np.ascontiguousarray(
            vq.reshape(cfg.NCHSUM, P, 4).transpose(1, 0, 2)
            .reshape(P, cfg.NCHSUM * 4).astype(BF)))
    return pc, pi


def _blockdiag_att(att, heads, hid, f):
    A = np.zeros((f, heads), dtype=np.float32)
    for h in range(heads):
        A[h * hid:(h + 1) * hid, h] = att[0, h]
    return A


def _ap(base, ap_list, off_extra=0):
    return bass.AP(tensor=base.tensor, offset=base.offset + off_extra,
                   ap=ap_list)


@with_exitstack
def _build_a(ctx, tc, cfg, t):
    nc = tc.nc
    TPC, H, HID, F = cfg.TPC, cfg.H, cfg.HID, cfg.F
    WCOLS = F + 2 * H              # 144
    TCOLS = F + H                  # 136 table cols used
    ROW = 256                      # table1 row elems (512B)
    W2COLS = cfg.OUT + 2

    consts = ctx.enter_context(tc.tile_pool(name="consts", bufs=1))
    wpack = consts.tile([P, WCOLS], BF16)
    nc.sync.dma_start(out=wpack[:], in_=t["wpack"][:, :])
    w2pack = consts.tile([P, W2COLS], BF16)
    nc.sync.dma_start(out=w2pack[:], in_=t["w2pack"][:, :])
    ident = consts.tile([P, P], F32)
    nc.sync.dma_start(out=ident[:], in_=t["ident"][:, :])
    s_sb = consts.tile([P, TPC * H], F32)   # own-tile s1, partition-aligned

    # ---------------- node phase ----------------
    NT = cfg.NTILES
    BLK = 8
    with ExitStack() as nctx:
        xpool = nctx.enter_context(tc.tile_pool(name="xt", bufs=2))
        npsum = nctx.enter_context(tc.tile_pool(name="npsum", bufs=2,
                                                space="PSUM"))
        nstage = nctx.enter_context(tc.tile_pool(name="nstage", bufs=3))
        for blk in range((NT + BLK - 1) // BLK):
            nt0 = blk * BLK
            nt1 = min(nt0 + BLK, NT)
            xt = xpool.tile([P, BLK * P], BF16, tag="xt")
            nc.sync.dma_start(out=xt[:, 0:(nt1 - nt0) * P],
                              in_=t["xT"][:, nt0 * P:nt1 * P])
            for j in range(nt1 - nt0):
                nt = nt0 + j
                pt = npsum.tile([P, WCOLS], F32, tag="npt")
                nc.tensor.matmul(out=pt[:], lhsT=xt[:, j * P:(j + 1) * P],
                                 rhs=wpack[:], start=True, stop=True)
                s1 = nstage.tile([P, TCOLS], BF16, tag="s1")
                nc.scalar.activation(s1[:], pt[:, 0:TCOLS], AF.Copy)
                nc.sync.dma_start(
                    out=t["table1"][nt * P:(nt + 1) * P, 0:TCOLS], in_=s1[:])
                if nt < TPC:   # own tiles are first in rotated order
                    nc.scalar.activation(s_sb[:, nt * H:(nt + 1) * H],
                                         pt[:, TCOLS:WCOLS], AF.Copy)

    tc.strict_bb_all_engine_barrier()

    # ---------------- edge phase ----------------
    ipool = ctx.enter_context(tc.tile_pool(name="idx", bufs=2))
    ppool = ctx.enter_context(tc.tile_pool(name="par", bufs=2))
    gpool = ctx.enter_context(tc.tile_pool(name="g", bufs=2))
    lpool = ctx.enter_context(tc.tile_pool(name="logit", bufs=2))
    mpool = ctx.enter_context(tc.tile_pool(name="msg", bufs=2))
    hpool = ctx.enter_context(tc.tile_pool(name="h1", bufs=2))
    tpsum = ctx.enter_context(tc.tile_pool(name="tpsum", bufs=2,
                                           space="PSUM"))
    t2psum = ctx.enter_context(tc.tile_pool(name="t2psum", bufs=2,
                                            space="PSUM"))

    tab_pair = _ap(t["table1"][:, :], [[2 * ROW, cfg.N_PAD // 2],
                                      [1, 2 * ROW]])
    c0 = 0
    for ti in range(TPC):
        NCH = cfg.NCH[ti]
        h0 = (NCH + 1) // 2
        # loads
        ia = ipool.tile([P, NCH * 8], I16, tag="ia")
        nc.sync.dma_start(out=ia[:], in_=t["idxA"][
            :, c0 * 8:(c0 + NCH) * 8])
        pe = ppool.tile([P, NCH], BF16, tag="pe")
        nc.sync.dma_start(out=pe[:], in_=t["pe"][:, c0:c0 + NCH])
        po = ppool.tile([P, NCH], BF16, tag="po")
        nc.sync.dma_start(out=po[:], in_=t["po"][:, c0:c0 + NCH])

        # gather (pair rows, 1024B elements) on 2 queues
        G = gpool.tile([P, NCH, 2 * ROW], BF16, tag="G")
        nc.gpsimd.dma_gather(
            out_ap=G[:, 0:h0, :], in_ap=tab_pair, idxs_ap=ia[:, 0:h0 * 8],
            num_idxs=h0 * P, num_idxs_reg=h0 * P, elem_size=2 * ROW,
            single_packet=False, queue_num=(2 * ti) % NQ)
        if NCH > h0:
            nc.gpsimd.dma_gather(
                out_ap=G[:, h0:NCH, :], in_ap=tab_pair,
                idxs_ap=ia[:, h0 * 8:NCH * 8],
                num_idxs=(NCH - h0) * P, num_idxs_reg=(NCH - h0) * P,
                elem_size=2 * ROW, single_packet=False,
                queue_num=(2 * ti + 1) % NQ)

        GP = G[:].ap[0]     # partition AP entry

        # d1 select: d1s = d1_even*pe + d1_odd*po
        t1 = lpool.tile([P, NCH, H], BF16, tag="t1")
        nc.vector.tensor_tensor(
            out=t1[:], in0=_ap(G[:], [GP, [2 * ROW, NCH], [1, H]], F),
            in1=_ap(pe[:], [pe[:].ap[0], [1, NCH], [0, H]]), op=OP.mult)
        t2_ = lpool.tile([P, NCH, H], BF16, tag="t2")
        nc.vector.tensor_tensor(
            out=t2_[:], in0=_ap(G[:], [GP, [2 * ROW, NCH], [1, H]], ROW + F),
            in1=_ap(po[:], [po[:].ap[0], [1, NCH], [0, H]]), op=OP.mult)
        d1s = lpool.tile([P, NCH, H], BF16, tag="d1s")
        nc.vector.tensor_tensor(out=d1s[:], in0=t1[:], in1=t2_[:], op=OP.add)
        # u = d1s + s1[dst] (partition-aligned broadcast over chunks)
        u = lpool.tile([P, NCH, H], BF16, tag="u")
        nc.vector.tensor_tensor(
            out=u[:], in0=d1s[:],
            in1=_ap(s_sb[:], [s_sb[:].ap[0], [0, NCH], [1, H]], ti * H),
            op=OP.add)
        a = lpool.tile([P, NCH, H], BF16, tag="a")
        nc.vector.scalar_tensor_tensor(out=a[:], in0=u[:], scalar=cfg.NEG,
                                       in1=u[:], op0=OP.mult, op1=OP.max)
        ex = lpool.tile([P, NCH, H], BF16, tag="ex")
        nc.scalar.activation(ex[:], a[:], AF.Exp)

        # we/wo [P, H, NCH] (transposed so den-reduce is innermost-contig)
        we = lpool.tile([P, H, NCH], BF16, tag="we")
        nc.vector.tensor_tensor(
            out=_ap(we[:], [we[:].ap[0], [1, NCH], [NCH, H]]),
            in0=ex[:],
            in1=_ap(pe[:], [pe[:].ap[0], [1, NCH], [0, H]]), op=OP.mult)
        wo = lpool.tile([P, H, NCH], BF16, tag="wo")
        nc.vector.tensor_tensor(
            out=_ap(wo[:], [wo[:].ap[0], [1, NCH], [NCH, H]]),
            in0=ex[:],
            in1=_ap(po[:], [po[:].ap[0], [1, NCH], [0, H]]), op=OP.mult)
        wsum = lpool.tile([P, H, NCH], F32, tag="wsum")
        nc.vector.tensor_tensor(out=wsum[:], in0=we[:], in1=wo[:], op=OP.add)
        den = hpool.tile([P, H], F32, tag="den")
        nc.vector.tensor_reduce(out=den[:], in_=wsum[:],
                                axis=mybir.AxisListType.X, op=OP.add)

        # messages: M[p, f, k] = h_row * w (f-major, k innermost); one M
        # buffer reused for even then odd pass to halve SBUF footprint.
        M = mpool.tile([P, F, NCH], BF16, tag="M")
        nc.vector.tensor_tensor(
            out=_ap(M[:], [M[:].ap[0], [1, NCH], [NCH * HID, H],
                           [NCH, HID]]),
            in0=_ap(G[:], [GP, [2 * ROW, NCH], [HID, H], [1, HID]]),
            in1=_ap(we[:], [we[:].ap[0], [1, NCH], [NCH, H], [0, HID]]),
            op=OP.mult)
        agg_e = hpool.tile([P, F], F32, tag="agg_e")
        nc.vector.tensor_reduce(out=agg_e[:], in_=M[:],
                                axis=mybir.AxisListType.X, op=OP.add)
        nc.vector.tensor_tensor(
            out=_ap(M[:], [M[:].ap[0], [1, NCH], [NCH * HID, H],
                           [NCH, HID]]),
            in0=_ap(G[:], [GP, [2 * ROW, NCH], [HID, H], [1, HID]], ROW),
            in1=_ap(wo[:], [wo[:].ap[0], [1, NCH], [NCH, H], [0, HID]]),
            op=OP.mult)
        agg_o = hpool.tile([P, F], F32, tag="agg_o")
        nc.vector.tensor_reduce(out=agg_o[:], in_=M[:],
                                axis=mybir.AxisListType.X, op=OP.add)
        agg = hpool.tile([P, F], F32, tag="agg")
        nc.vector.tensor_tensor(out=agg[:], in0=agg_e[:], in1=agg_o[:],
                                op=OP.add)

        # normalize + elu + layer2 node transform
        dene = hpool.tile([P, H], F32, tag="dene")
        nc.vector.tensor_scalar_add(dene[:], den[:], 1e-20)
        rcp = hpool.tile([P, H], F32, tag="rcp")
        nc.vector.reciprocal(rcp[:], dene[:])
        h1 = hpool.tile([P, F], F32, tag="h1")
        nc.vector.tensor_tensor(
            out=_ap(h1[:], [h1[:].ap[0], [HID, H], [1, HID]]),
            in0=_ap(agg[:], [agg[:].ap[0], [HID, H], [1, HID]]),
            in1=_ap(rcp[:], [rcp[:].ap[0], [1, H], [0, HID]]),
            op=OP.mult)
        neg = hpool.tile([P, F], F32, tag="neg")
        nc.vector.tensor_scalar_min(neg[:], h1[:], 0.0)
        pos = hpool.tile([P, F], F32, tag="pos")
        nc.vector.tensor_scalar_max(pos[:], h1[:], 0.0)
        een = hpool.tile([P, F], F32, tag="een")
        nc.scalar.activation(een[:], neg[:], AF.Exp)
        elu = hpool.tile([P, F], F32, tag="elu")
        nc.vector.scalar_tensor_tensor(out=elu[:], in0=een[:], scalar=-1.0,
                                       in1=pos[:], op0=OP.add, op1=OP.add)
        eT_ps = tpsum.tile([P, P], F32, tag="eT")
        nc.tensor.transpose(out=eT_ps[:], in_=elu[:], identity=ident[:])
        eT = hpool.tile([P, P], BF16, tag="eTs")
        nc.scalar.activation(eT[:], eT_ps[:], AF.Copy)
        t2p = t2psum.tile([P, W2COLS], F32, tag="t2")
        nc.tensor.matmul(out=t2p[:], lhsT=eT[:], rhs=w2pack[:],
                         start=True, stop=True)
        t2s = hpool.tile([P, W2COLS], F32, tag="t2s")
        nc.scalar.activation(t2s[:], t2p[:], AF.Copy)
        nc.sync.dma_start(out=t["slab"][ti * P:(ti + 1) * P, :], in_=t2s[:])
        c0 += NCH


@with_exitstack
def _build_b(ctx, tc, cfg, t):
    nc = tc.nc
    TPC, OUT = cfg.TPC, cfg.OUT
    QROW = 32          # table2 row elems (64B), 4 rows per 256B element

    consts = ctx.enter_context(tc.tile_pool(name="consts", bufs=1))
    s2sb = consts.tile([P, TPC], F32)
    nc.sync.dma_start(out=s2sb[:], in_=t["s2sb"][:, :])

    ipool = ctx.enter_context(tc.tile_pool(name="idx", bufs=2))
    qpool = ctx.enter_context(tc.tile_pool(name="mq", bufs=2))
    gpool = ctx.enter_context(tc.tile_pool(name="g2", bufs=3))
    lpool = ctx.enter_context(tc.tile_pool(name="l2", bufs=2))
    mpool = ctx.enter_context(tc.tile_pool(name="m2", bufs=2))
    opool = ctx.enter_context(tc.tile_pool(name="o", bufs=3))

    tab_q = _ap(t["table2"][:, :], [[4 * QROW, cfg.N_PAD // 4],
                                    [1, 4 * QROW]])
    c0 = 0
    for ti in range(TPC):
        NCH = cfg.NCH[ti]
        h0 = (NCH + 1) // 2
        ib = ipool.tile([P, NCH * 8], I16, tag="ib")
        nc.sync.dma_start(out=ib[:], in_=t["idxB"][
            :, c0 * 8:(c0 + NCH) * 8])
        mq = qpool.tile([P, NCH, 4], BF16, tag="mq")
        nc.sync.dma_start(out=mq[:], in_=t["mq4"][:, c0 * 4:(c0 + NCH) * 4])

        G2 = gpool.tile([P, NCH, 4 * QROW], BF16, tag="G2")
        nc.gpsimd.dma_gather(
            out_ap=G2[:, 0:h0, :], in_ap=tab_q, idxs_ap=ib[:, 0:h0 * 8],
            num_idxs=h0 * P, num_idxs_reg=h0 * P, elem_size=4 * QROW,
            single_packet=False, queue_num=(2 * ti) % NQ)
        if NCH > h0:
            nc.gpsimd.dma_gather(
                out_ap=G2[:, h0:NCH, :], in_ap=tab_q,
                idxs_ap=ib[:, h0 * 8:NCH * 8],
                num_idxs=(NCH - h0) * P, num_idxs_reg=(NCH - h0) * P,
                elem_size=4 * QROW, single_packet=False,
                queue_num=(2 * ti + 1) % NQ)

        GP = G2[:].ap[0]
        # d2 select: sum over quad of d2_q * mq
        dq = lpool.tile([P, NCH, 4], F32, tag="dq")
        nc.vector.tensor_tensor(
            out=dq[:], in0=_ap(G2[:], [GP, [4 * QROW, NCH], [QROW, 4]], OUT),
            in1=mq[:], op=OP.mult)
        d2s = lpool.tile([P, NCH], F32, tag="d2s")
        nc.vector.tensor_reduce(out=d2s[:], in_=dq[:],
                                axis=mybir.AxisListType.X, op=OP.add)
        u = lpool.tile([P, NCH], F32, tag="u2")
        nc.vector.tensor_tensor(
            out=u[:], in0=d2s[:],
            in1=_ap(s2sb[:], [s2sb[:].ap[0], [0, NCH]], ti), op=OP.add)
        a = lpool.tile([P, NCH], F32, tag="a2")
        nc.vector.scalar_tensor_tensor(out=a[:], in0=u[:], scalar=cfg.NEG,
                                       in1=u[:], op0=OP.mult, op1=OP.max)
        ex = lpool.tile([P, NCH], BF16, tag="ex2")
        nc.scalar.activation(ex[:], a[:], AF.Exp)
        w4 = lpool.tile([P, NCH, 4], BF16, tag="w4")
        nc.vector.tensor_tensor(
            out=w4[:], in0=mq[:],
            in1=_ap(ex[:], [ex[:].ap[0], [1, NCH], [0, 4]]), op=OP.mult)
        den = opool.tile([P, 1], F32, tag="den2")
        nc.vector.tensor_reduce(
            out=den[:], in_=_ap(w4[:], [w4[:].ap[0], [1, 4 * NCH]]),
            axis=mybir.AxisListType.X, op=OP.add)

        # messages M4[p, f, 4k+q] = feat_q[f] * w4[q]
        M4 = mpool.tile([P, OUT, 4 * NCH], BF16, tag="M4")
        for q in range(4):
            nc.vector.tensor_tensor(
                out=_ap(M4[:], [M4[:].ap[0], [4, NCH], [4 * NCH, OUT]], q),
                in0=_ap(G2[:], [GP, [4 * QROW, NCH], [1, OUT]], q * QROW),
                in1=_ap(w4[:], [w4[:].ap[0], [4, NCH], [0, OUT]], q),
                op=OP.mult)
        agg = opool.tile([P, OUT], F32, tag="agg2")
        nc.vector.tensor_reduce(out=agg[:], in_=M4[:],
                                axis=mybir.AxisListType.X, op=OP.add)

        dene = opool.tile([P, 1], F32, tag="dene")
        nc.vector.tensor_scalar_add(dene[:], den[:], 1e-20)
        rcp = opool.tile([P, 1], F32, tag="rcp")
        nc.vector.reciprocal(rcp[:], dene[:])
        h2 = opool.tile([P, OUT], F32, tag="h2")
        nc.vector.tensor_tensor(
            out=h2[:], in0=agg[:],
            in1=_ap(rcp[:], [rcp[:].ap[0], [0, OUT]]), op=OP.mult)
        m = opool.tile([P, 1], F32, tag="m")
        nc.vector.tensor_reduce(out=m[:], in_=h2[:],
                                axis=mybir.AxisListType.X, op=OP.max)
        tm = opool.tile([P, OUT], F32, tag="tm")
        nc.vector.tensor_tensor(
            out=tm[:], in0=h2[:],
            in1=_ap(m[:], [m[:].ap[0], [0, OUT]]), op=OP.subtract)
        pex = opool.tile([P, OUT], F32, tag="pex")
        ssum = opool.tile([P, 1], F32, tag="ss")
        nc.scalar.activation(pex[:], tm[:], AF.Exp, accum_out=ssum[:])
        ln = opool.tile([P, 1], F32, tag="ln")
        nc.scalar.activation(ln[:], ssum[:], AF.Ln)
        res = opool.tile([P, OUT], F32, tag="res")
        nc.vector.tensor_tensor(
            out=res[:], in0=tm[:],
            in1=_ap(ln[:], [ln[:].ap[0], [0, OUT]]), op=OP.subtract)
        nc.sync.dma_start(out=t["outp"][ti * P:(ti + 1) * P, :], in_=res[:])
        c0 += NCH


def _decl_a(nc, cfg):
    t = {}
    WCOLS = cfg.F + 2 * cfg.H
    W2COLS = cfg.OUT + 2

    def inp(name, shape, dt):
        t[name] = nc.dram_tensor(name, shape, dt, kind="ExternalInput").ap()

    inp("xT", [P, cfg.N_PAD], BF16)
    inp("wpack", [P, WCOLS], BF16)
    inp("w2pack", [P, W2COLS], BF16)
    inp("ident", [P, P], F32)
    inp("idxA", [P, cfg.NCHSUM * 8], I16)
    inp("pe", [P, cfg.NCHSUM], BF16)
    inp("po", [P, cfg.NCHSUM], BF16)
    t["table1"] = nc.dram_tensor("table1", [cfg.N_PAD, 256], BF16,
                                 kind="Internal").ap()
    t["slab"] = nc.dram_tensor("slab", [cfg.NPC_PAD, W2COLS], F32,
                               kind="ExternalOutput").ap()
    return t


def _decl_b(nc, cfg):
    t = {}

    def inp(name, shape, dt):
        t[name] = nc.dram_tensor(name, shape, dt, kind="ExternalInput").ap()

    inp("table2", [cfg.N_PAD, 32], BF16)
    inp("s2sb", [P, cfg.TPC], F32)
    inp("idxB", [P, cfg.NCHSUM * 8], I16)
    inp("mq4", [P, cfg.NCHSUM * 4], BF16)
    t["outp"] = nc.dram_tensor("outp", [cfg.NPC_PAD, cfg.OUT], F32,
                               kind="ExternalOutput").ap()
    return t


def _compile(build_fn, decl_fn, cfg):
    nc = bacc.Bacc("TRN2", target_bir_lowering=False, debug=False,
                   enable_asserts=False, num_devices=cfg.NCORES,
                   num_swdge_queues=NQ)
    t = decl_fn(nc, cfg)
    with tile.TileContext(nc) as tc:
        build_fn(tc, cfg, t)
    nc.compile()
    return nc


def _host_prep_weights(cfg, W1, att_src1, att_dst1, W2, att_src2, att_dst2):
    A_d1 = _blockdiag_att(np.asarray(att_dst1, np.float32), cfg.H, cfg.HID,
                          cfg.F)
    A_s1 = _blockdiag_att(np.asarray(att_src1, np.float32), cfg.H, cfg.HID,
                          cfg.F)
    W1T = np.asarray(W1, np.float32).T.copy()
    wpack = np.concatenate([W1T, W1T @ A_d1, W1T @ A_s1], axis=1)
    W2T = np.asarray(W2, np.float32).T.copy()
    a_d2 = np.asarray(att_dst2, np.float32).reshape(cfg.OUT, 1)
    a_s2 = np.asarray(att_src2, np.float32).reshape(cfg.OUT, 1)
    w2pack = np.concatenate([W2T, W2T @ a_d2, W2T @ a_s2], axis=1)
    return wpack.astype(BF), w2pack.astype(BF)


_CACHE = {}


def _get_kernels(cfg):
    key = (cfg.N, cfg.E, cfg.NCORES, tuple(cfg.NCH))
    if key not in _CACHE:
        nca = _compile(_build_a, _decl_a, cfg)
        ncb = _compile(_build_b, _decl_b, cfg)
        _CACHE[key] = (nca, ncb)
    return _CACHE[key]


def run(cfg, inputs, runner=None):
    x = np.asarray(inputs["x"], np.float32)
    edge_index = np.asarray(inputs["edge_index"], np.int64)
    pc, pi = _prep_graph(cfg, edge_index)
    wpack, w2pack = _host_prep_weights(
        cfg, inputs["W1"], inputs["att_src1"], inputs["att_dst1"],
        inputs["W2"], inputs["att_src2"], inputs["att_dst2"])

    x_pi = np.zeros((cfg.N_PAD, cfg.IN), np.float32)
    x_pi[pi] = x
    ident = np.eye(P, dtype=np.float32)

    nca, ncb = _get_kernels(cfg)

    if runner is None:
        def runner(nc, in_maps):
            r = bass_utils.run_bass_kernel_spmd(
                nc, in_maps, core_ids=list(range(cfg.NCORES)))
            return r.results

    in_maps_a = []
    for c in range(cfg.NCORES):
        x_rot = np.roll(x_pi, -c * cfg.NPC_PAD, axis=0)
        in_maps_a.append(dict(
            xT=np.ascontiguousarray(x_rot.T).astype(BF), wpack=wpack,
            w2pack=w2pack, ident=ident, idxA=pc["idxA"][c],
            pe=pc["pe"][c], po=pc["po"][c]))
    res_a = runner(nca, in_maps_a)

    # host repack: slab (feat2|d2|s2 per own node, pi order) -> table2
    W2C = cfg.OUT + 2
    slab_pi = np.zeros((cfg.N_PAD, W2C), np.float32)
    for c in range(cfg.NCORES):
        slab_pi[c * cfg.NPC_PAD:(c + 1) * cfg.NPC_PAD] = \
            np.asarray(res_a[c]["slab"], np.float32)
    tab2_pi = np.zeros((cfg.N_PAD, 32), BF)
    tab2_pi[:, 0:cfg.OUT + 1] = slab_pi[:, 0:cfg.OUT + 1].astype(BF)

    in_maps_b = []
    for c in range(cfg.NCORES):
        tab2_rot = np.ascontiguousarray(
            np.roll(tab2_pi, -c * cfg.NPC_PAD, axis=0))
        s2sb = np.ascontiguousarray(
            slab_pi[c * cfg.NPC_PAD:(c + 1) * cfg.NPC_PAD, cfg.OUT + 1]
            .reshape(cfg.TPC, P).T).astype(np.float32)
        in_maps_b.append(dict(
            table2=tab2_rot, s2sb=s2sb, idxB=pc["idxB"][c],
            mq4=pc["mq4"][c]))
    res_b = runner(ncb, in_maps_b)

    out_pi = np.zeros((cfg.N_PAD, cfg.OUT), np.float32)
    for c in range(cfg.NCORES):
        out_pi[c * cfg.NPC_PAD:(c + 1) * cfg.NPC_PAD] = \
            np.asarray(res_b[c]["outp"], np.float32)
    return out_pi[pi]


def kernel(**inputs):
    cfg = Cfg(N=50000, E=1600000, ncores=8)
    return run(cfg, inputs)


# revision 18
# speedup vs baseline: 3.5875x; 2.3294x over previous
"""
2-layer GAT on Trainium2 (8 NeuronCores, SPMD via bass/Tile) — v2.

Design notes (what makes this fast):

The dominant cost on TRN2 for random-edge GNN aggregation is SWDGE
descriptor generation for dma_gather: ~8ns per gathered element on one
queue, measured.  So the kernel is built around exactly ONE gather
element per edge per layer, generated on 4 parallel SWDGE queues
(~2ns/elem):

- Nodes are permuted (host-side, from edge_index only): each core's dst
  nodes are sorted by in-degree (desc) and tiled 128 at a time.  Within
  a tile every dst sits at a fixed partition; edge slot (p, k) holds the
  k-th in-edge of the p-th dst.  Degree sorting makes per-tile max
  degree ~= mean degree, so slot padding is only a few %.
- Because slot partition == dst, aggregation is a plain free-axis
  reduce (DVE), the dst-side attention term is partition-aligned (no
  per-edge gather/select for it), and no one-hot matrices exist at all.
- The int16 gather-index limit (N=50176 > 32767) is dodged by gathering
  PAIRS of table rows (idx = row>>1, 1024B element) and resolving row
  parity on DVE with host-streamed (1-par)/par weights, fused into the
  message-scaling multiply.
- Layer tables are written in a per-core ROTATED row order (own nodes
  first) so the same SPMD program can stash the dst-side attention
  columns for its own tiles at compile-time offsets.
- Between the two layer kernels the host only repacks device-computed
  numbers (layer-2 node table) — all FLOPs happen on device.
"""

import os
import sys
from contextlib import ExitStack

import numpy as np
import ml_dtypes

for _p in ("/opt/trn_rl_repo",):
    if os.path.isdir(_p) and _p not in sys.path:
        sys.path.insert(0, _p)

import concourse.bass as bass
import concourse.bacc as bacc
import concourse.tile as tile
from concourse import mybir
from concourse import bass_utils
from concourse._compat import with_exitstack

F32 = mybir.dt.float32
BF16 = mybir.dt.bfloat16
I16 = mybir.dt.int16
AF = mybir.ActivationFunctionType
OP = mybir.AluOpType
P = 128
BF = ml_dtypes.bfloat16
NQ = 4          # SWDGE queues


class Cfg:
    def __init__(self, N, E, ncores, neg=0.2, in_ch=128, f=128, heads=8,
                 hid=16, out=16):
        self.N = N
        self.E = E
        self.NCORES = ncores
        self.NEG = neg
        self.IN = in_ch
        self.F = f
        self.H = heads
        self.HID = hid
        self.OUT = out
        self.NPC = N // ncores                    # 6250
        self.TPC = (self.NPC + P - 1) // P        # 49
        self.NPC_PAD = self.TPC * P               # 6272
        self.N_PAD = ncores * self.NPC_PAD        # 50176
        self.NTILES = ncores * self.TPC           # 392
        self.NCH = None       # [TPC] chunks per tile (shared across cores)
        self.NCHSUM = None    # sum(NCH)


def _wrap16(vals):
    """[n] slot-ordered int values -> [128, n//16] int16 wrapped layout."""
    n = vals.shape[0]
    assert n % 16 == 0
    w = vals.reshape(-1, 16).T.astype(np.int16)
    return np.ascontiguousarray(np.tile(w, (8, 1)))


def _prep_graph(cfg, edge_index):
    """Degree-sorted slot layout. Returns per-core arrays + permutation."""
    N, NPC, NPC_PAD, TPC, C = cfg.N, cfg.NPC, cfg.NPC_PAD, cfg.TPC, cfg.NCORES
    src = np.concatenate([edge_index[0], np.arange(N, dtype=np.int64)])
    dst = np.concatenate([edge_index[1], np.arange(N, dtype=np.int64)])

    deg = np.bincount(dst, minlength=N)
    core = dst // NPC

    # permutation: within each core, nodes sorted by degree desc
    pi = np.empty(N, np.int64)           # old node -> pi row
    for c in range(C):
        nodes = np.arange(c * NPC, (c + 1) * NPC)
        order = nodes[np.argsort(-deg[nodes], kind="stable")]
        pi[order] = c * NPC_PAD + np.arange(NPC)

    rank = pi[dst] - core * NPC_PAD      # 0..NPC-1 within core
    tile_id = rank // P
    p_part = rank % P

    # edge rank k within its dst (order of appearance)
    order = np.lexsort((src, dst))
    k_sorted = np.arange(len(dst)) - np.repeat(
        np.concatenate([[0], np.cumsum(np.bincount(dst, minlength=N))[:-1]]),
        np.bincount(dst, minlength=N))
    k = np.empty(len(dst), np.int64)
    k[order] = k_sorted

    # chunks per tile: max over cores of per-(core,tile) max degree
    mx = np.zeros(C * TPC, np.int64)
    np.maximum.at(mx, core * TPC + tile_id, k + 1)
    NCH = np.maximum(mx.reshape(C, TPC).max(axis=0), 1)
    cfg.NCH = [int(x) for x in NCH]
    cfg.NCHSUM = int(NCH.sum())
    tile_base = np.concatenate([[0], np.cumsum(NCH)])   # in chunks

    pis = pi[src]
    pc = dict(idxA=[], idxB=[], pe=[], po=[], mq4=[])
    for c in range(C):
        m = core == c
        rot = (pis[m] - c * NPC_PAD) % cfg.N_PAD
        slot = (tile_base[tile_id[m]] + k[m]) * P + p_part[m]
        ns = cfg.NCHSUM * P
        vA = np.zeros(ns, np.int64)
        vB = np.zeros(ns, np.int64)
        vpe = np.zeros(ns, np.float32)
        vpo = np.zeros(ns, np.float32)
        vq = np.zeros((ns, 4), np.float32)
        vA[slot] = rot >> 1
        vB[slot] = rot >> 2
        par = rot & 1
        vpe[slot] = 1.0 - par
        vpo[slot] = par
        vq[slot, rot & 3] = 1.0
        pc["idxA"].append(_wrap16(vA))
        pc["idxB"].append(_wrap16(vB))
        # [128, NCHSUM] layouts (partition = slot p, col = chunk)
        pc["pe"].append(np.ascontiguousarray(
            vpe.reshape(cfg.NCHSUM, P).T.astype(BF)))
        pc["po"].append(np.ascontiguousarray(
            vpo.reshape(cfg.NCHSUM, P).T.astype(BF)))
        # [128, NCHSUM*4]
        pc["mq4"].append(np.ascontiguousarray(
            vq.reshape(cfg.NCHSUM, P, 4).transpose(1, 0, 2)
            .reshape(P, cfg.NCHSUM * 4).astype(BF)))
    return pc, pi


def _blockdiag_att(att, heads, hid, f):
    A = np.zeros((f, heads), dtype=np.float32)
    for h in range(heads):
        A[h * hid:(h + 1) * hid, h] = att[0, h]
    return A


def _ap(base, ap_list, off_extra=0):
    return bass.AP(tensor=base.tensor, offset=base.offset + off_extra,
                   ap=ap_list)


@with_exitstack
def _build_a(ctx, tc, cfg, t):
    nc = tc.nc
    TPC, H, HID, F = cfg.TPC, cfg.H, cfg.HID, cfg.F
    WCOLS = F + 2 * H              # 144
    TCOLS = F + H                  # 136 table cols used
    ROW = 256                      # table1 row elems (512B)
    W2COLS = cfg.OUT + 2

    consts = ctx.enter_context(tc.tile_pool(name="consts", bufs=1))
    wpack = consts.tile([P, WCOLS], BF16)
    nc.sync.dma_start(out=wpack[:], in_=t["wpack"][:, :])
    w2pack = consts.tile([P, W2COLS], BF16)
    nc.sync.dma_start(out=w2pack[:], in_=t["w2pack"][:, :])
    ident = consts.tile([P, P], F32)
    nc.sync.dma_start(out=ident[:], in_=t["ident"][:, :])
    s_sb = consts.tile([P, TPC * H], F32)   # own-tile s1, partition-aligned

    # ---------------- node phase ----------------
    NT = cfg.NTILES
    BLK = 8
    with ExitStack() as nctx:
        xpool = nctx.enter_context(tc.tile_pool(name="xt", bufs=2))
        npsum = nctx.enter_context(tc.tile_pool(name="npsum", bufs=2,
                                                space="PSUM"))
        nstage = nctx.enter_context(tc.tile_pool(name="nstage", bufs=3))
        for blk in range((NT + BLK - 1) // BLK):
            nt0 = blk * BLK
            nt1 = min(nt0 + BLK, NT)
            xt = xpool.tile([P, BLK * P], BF16, tag="xt")
            nc.sync.dma_start(out=xt[:, 0:(nt1 - nt0) * P],
                              in_=t["xT"][:, nt0 * P:nt1 * P])
            for j in range(nt1 - nt0):
                nt = nt0 + j
                pt = npsum.tile([P, WCOLS], F32, tag="npt")
                nc.tensor.matmul(out=pt[:], lhsT=xt[:, j * P:(j + 1) * P],
                                 rhs=wpack[:], start=True, stop=True)
                # stage a FULL 512B row (cols 136:256 stale garbage, never
                # read) so the table write is one contiguous 64KB DMA.
                s1 = nstage.tile([P, ROW], BF16, tag="s1")
                nc.scalar.activation(s1[:, 0:TCOLS], pt[:, 0:TCOLS], AF.Copy)
                nc.sync.dma_start(
                    out=t["table1"][nt * P:(nt + 1) * P, :], in_=s1[:])
                if nt < TPC:   # own tiles are first in rotated order
                    nc.scalar.activation(s_sb[:, nt * H:(nt + 1) * H],
                                         pt[:, TCOLS:WCOLS], AF.Copy)

    tc.strict_bb_all_engine_barrier()

    # ---------------- edge phase ----------------
    ipool = ctx.enter_context(tc.tile_pool(name="idx", bufs=2))
    ppool = ctx.enter_context(tc.tile_pool(name="par", bufs=2))
    gpool = ctx.enter_context(tc.tile_pool(name="g", bufs=2))
    lpool = ctx.enter_context(tc.tile_pool(name="logit", bufs=2))
    mpool = ctx.enter_context(tc.tile_pool(name="msg", bufs=2))
    hpool = ctx.enter_context(tc.tile_pool(name="h1", bufs=2))
    tpsum = ctx.enter_context(tc.tile_pool(name="tpsum", bufs=2,
                                           space="PSUM"))
    t2psum = ctx.enter_context(tc.tile_pool(name="t2psum", bufs=2,
                                            space="PSUM"))

    tab_pair = _ap(t["table1"][:, :], [[2 * ROW, cfg.N_PAD // 2],
                                      [1, 2 * ROW]])
    c0 = 0
    for ti in range(TPC):
        NCH = cfg.NCH[ti]
        # loads
        ia = ipool.tile([P, NCH * 8], I16, tag="ia")
        nc.sync.dma_start(out=ia[:], in_=t["idxA"][
            :, c0 * 8:(c0 + NCH) * 8])
        pe = ppool.tile([P, NCH], BF16, tag="pe")
        nc.sync.dma_start(out=pe[:], in_=t["pe"][:, c0:c0 + NCH])
        po = ppool.tile([P, NCH], BF16, tag="po")
        nc.sync.dma_start(out=po[:], in_=t["po"][:, c0:c0 + NCH])

        # gather (pair rows, 1024B elements) on all 4 SWDGE queues
        G = gpool.tile([P, NCH, 2 * ROW], BF16, tag="G")
        for j in range(NQ):
            a0 = NCH * j // NQ
            a1 = NCH * (j + 1) // NQ
            if a1 > a0:
                nc.gpsimd.dma_gather(
                    out_ap=G[:, a0:a1, :], in_ap=tab_pair,
                    idxs_ap=ia[:, a0 * 8:a1 * 8],
                    num_idxs=(a1 - a0) * P, num_idxs_reg=(a1 - a0) * P,
                    elem_size=2 * ROW, single_packet=False, queue_num=j)

        GP = G[:].ap[0]     # partition AP entry

        # d1 select: d1s = d1_even*pe + d1_odd*po
        t1 = lpool.tile([P, NCH, H], BF16, tag="t1")
        nc.vector.tensor_tensor(
            out=t1[:], in0=_ap(G[:], [GP, [2 * ROW, NCH], [1, H]], F),
            in1=_ap(pe[:], [pe[:].ap[0], [1, NCH], [0, H]]), op=OP.mult)
        t2_ = lpool.tile([P, NCH, H], BF16, tag="t2")
        nc.vector.tensor_tensor(
            out=t2_[:], in0=_ap(G[:], [GP, [2 * ROW, NCH], [1, H]], ROW + F),
            in1=_ap(po[:], [po[:].ap[0], [1, NCH], [0, H]]), op=OP.mult)
        d1s = lpool.tile([P, NCH, H], BF16, tag="d1s")
        nc.vector.tensor_tensor(out=d1s[:], in0=t1[:], in1=t2_[:], op=OP.add)
        # u = d1s + s1[dst] (partition-aligned broadcast over chunks)
        u = lpool.tile([P, NCH, H], BF16, tag="u")
        nc.vector.tensor_tensor(
            out=u[:], in0=d1s[:],
            in1=_ap(s_sb[:], [s_sb[:].ap[0], [0, NCH], [1, H]], ti * H),
            op=OP.add)
        a = lpool.tile([P, NCH, H], BF16, tag="a")
        nc.vector.scalar_tensor_tensor(out=a[:], in0=u[:], scalar=cfg.NEG,
                                       in1=u[:], op0=OP.mult, op1=OP.max)
        ex = lpool.tile([P, NCH, H], BF16, tag="ex")
        nc.scalar.activation(ex[:], a[:], AF.Exp)

        # we/wo [P, NCH, H] k-major (all ops contiguous-innermost)
        we = lpool.tile([P, NCH, H], BF16, tag="we")
        nc.vector.tensor_tensor(
            out=we[:], in0=ex[:],
            in1=_ap(pe[:], [pe[:].ap[0], [1, NCH], [0, H]]), op=OP.mult)
        wo = lpool.tile([P, NCH, H], BF16, tag="wo")
        nc.vector.tensor_tensor(
            out=wo[:], in0=ex[:],
            in1=_ap(po[:], [po[:].ap[0], [1, NCH], [0, H]]), op=OP.mult)
        wsum = lpool.tile([P, NCH, H], F32, tag="wsum")
        nc.vector.tensor_tensor(out=wsum[:], in0=we[:], in1=wo[:], op=OP.add)
        # den: in-place tree over the chunk axis (contiguous adds)
        n = NCH
        while n > 1:
            h_ = n // 2
            nc.vector.tensor_tensor(out=wsum[:, 0:h_, :],
                                    in0=wsum[:, 0:h_, :],
                                    in1=wsum[:, n - h_:n, :], op=OP.add)
            n -= h_
        dene = hpool.tile([P, H], F32, tag="dene")
        nc.vector.tensor_scalar_add(dene[:], wsum[:, 0, :], 1e-20)

        # messages: M2[p, 2k(+1), f] = h_pair * w  (k-major, contiguous)
        M2 = mpool.tile([P, 2 * NCH, F], BF16, tag="M2", bufs=1)
        nc.vector.tensor_tensor(
            out=_ap(M2[:], [M2[:].ap[0], [F, NCH], [HID, H], [1, HID]]),
            in0=_ap(G[:], [GP, [2 * ROW, NCH], [HID, H], [1, HID]]),
            in1=_ap(we[:], [we[:].ap[0], [H, NCH], [1, H], [0, HID]]),
            op=OP.mult)
        nc.vector.tensor_tensor(
            out=_ap(M2[:], [M2[:].ap[0], [F, NCH], [HID, H], [1, HID]],
                    NCH * F),
            in0=_ap(G[:], [GP, [2 * ROW, NCH], [HID, H], [1, HID]], ROW),
            in1=_ap(wo[:], [wo[:].ap[0], [H, NCH], [1, H], [0, HID]]),
            op=OP.mult)
        # agg: in-place bf16 tree down to 4 chunks, then f32 finish
        n = 2 * NCH
        while n > 4:
            h_ = n // 2
            nc.vector.tensor_tensor(out=M2[:, 0:h_, :], in0=M2[:, 0:h_, :],
                                    in1=M2[:, n - h_:n, :], op=OP.add)
            n -= h_
        agg_a = hpool.tile([P, F], F32, tag="agg_a")
        nc.vector.tensor_tensor(out=agg_a[:], in0=M2[:, 0, :],
                                in1=M2[:, 1, :], op=OP.add)
        agg_b = hpool.tile([P, F], F32, tag="agg_b")
        if n == 4:
            nc.vector.tensor_tensor(out=agg_b[:], in0=M2[:, 2, :],
                                    in1=M2[:, 3, :], op=OP.add)
        elif n == 3:
            nc.vector.tensor_copy(agg_b[:], M2[:, 2, :])
        else:
            nc.vector.memset(agg_b[:], 0.0)
        agg = hpool.tile([P, F], F32, tag="agg")
        nc.vector.tensor_tensor(out=agg[:], in0=agg_a[:], in1=agg_b[:],
                                op=OP.add)

        # normalize + elu + layer2 node transform
        rcp = hpool.tile([P, H], F32, tag="rcp")
        nc.vector.reciprocal(rcp[:], dene[:])
        h1 = hpool.tile([P, F], F32, tag="h1")
        nc.vector.tensor_tensor(
            out=_ap(h1[:], [h1[:].ap[0], [HID, H], [1, HID]]),
            in0=_ap(agg[:], [agg[:].ap[0], [HID, H], [1, HID]]),
            in1=_ap(rcp[:], [rcp[:].ap[0], [1, H], [0, HID]]),
            op=OP.mult)
        neg = hpool.tile([P, F], F32, tag="neg")
        nc.vector.tensor_scalar_min(neg[:], h1[:], 0.0)
        pos = hpool.tile([P, F], F32, tag="pos")
        nc.vector.tensor_scalar_max(pos[:], h1[:], 0.0)
        een = hpool.tile([P, F], F32, tag="een")
        nc.scalar.activation(een[:], neg[:], AF.Exp)
        elu = hpool.tile([P, F], F32, tag="elu")
        nc.vector.scalar_tensor_tensor(out=elu[:], in0=een[:], scalar=-1.0,
                                       in1=pos[:], op0=OP.add, op1=OP.add)
        eT_ps = tpsum.tile([P, P], F32, tag="eT")
        nc.tensor.transpose(out=eT_ps[:], in_=elu[:], identity=ident[:])
        eT = hpool.tile([P, P], BF16, tag="eTs")
        nc.scalar.activation(eT[:], eT_ps[:], AF.Copy)
        t2p = t2psum.tile([P, W2COLS], F32, tag="t2")
        nc.tensor.matmul(out=t2p[:], lhsT=eT[:], rhs=w2pack[:],
                         start=True, stop=True)
        t2s = hpool.tile([P, W2COLS], F32, tag="t2s")
        nc.scalar.activation(t2s[:], t2p[:], AF.Copy)
        nc.sync.dma_start(out=t["slab"][ti * P:(ti + 1) * P, :], in_=t2s[:])
        c0 += NCH


@with_exitstack
def _build_b(ctx, tc, cfg, t):
    nc = tc.nc
    TPC, OUT = cfg.TPC, cfg.OUT
    QROW = 32          # table2 row elems (64B), 4 rows per 256B element

    consts = ctx.enter_context(tc.tile_pool(name="consts", bufs=1))
    s2sb = consts.tile([P, TPC], F32)
    nc.sync.dma_start(out=s2sb[:], in_=t["s2sb"][:, :])

    ipool = ctx.enter_context(tc.tile_pool(name="idx", bufs=2))
    qpool = ctx.enter_context(tc.tile_pool(name="mq", bufs=2))
    gpool = ctx.enter_context(tc.tile_pool(name="g2", bufs=3))
    lpool = ctx.enter_context(tc.tile_pool(name="l2", bufs=2))
    mpool = ctx.enter_context(tc.tile_pool(name="m2", bufs=2))
    opool = ctx.enter_context(tc.tile_pool(name="o", bufs=3))

    tab_q = _ap(t["table2"][:, :], [[4 * QROW, cfg.N_PAD // 4],
                                    [1, 4 * QROW]])
    c0 = 0
    for ti in range(TPC):
        NCH = cfg.NCH[ti]
        ib = ipool.tile([P, NCH * 8], I16, tag="ib")
        nc.sync.dma_start(out=ib[:], in_=t["idxB"][
            :, c0 * 8:(c0 + NCH) * 8])
        mq = qpool.tile([P, NCH, 4], BF16, tag="mq")
        nc.sync.dma_start(out=mq[:], in_=t["mq4"][:, c0 * 4:(c0 + NCH) * 4])

        G2 = gpool.tile([P, NCH, 4 * QROW], BF16, tag="G2")
        for j in range(NQ):
            a0 = NCH * j // NQ
            a1 = NCH * (j + 1) // NQ
            if a1 > a0:
                nc.gpsimd.dma_gather(
                    out_ap=G2[:, a0:a1, :], in_ap=tab_q,
                    idxs_ap=ib[:, a0 * 8:a1 * 8],
                    num_idxs=(a1 - a0) * P, num_idxs_reg=(a1 - a0) * P,
                    elem_size=4 * QROW, single_packet=False, queue_num=j)

        GP = G2[:].ap[0]
        # d2 select: sum over quad of d2_q * mq
        dq = lpool.tile([P, NCH, 4], F32, tag="dq")
        nc.vector.tensor_tensor(
            out=dq[:], in0=_ap(G2[:], [GP, [4 * QROW, NCH], [QROW, 4]], OUT),
            in1=mq[:], op=OP.mult)
        d2s = lpool.tile([P, NCH], F32, tag="d2s")
        nc.vector.tensor_reduce(out=d2s[:], in_=dq[:],
                                axis=mybir.AxisListType.X, op=OP.add)
        u = lpool.tile([P, NCH], F32, tag="u2")
        nc.vector.tensor_tensor(
            out=u[:], in0=d2s[:],
            in1=_ap(s2sb[:], [s2sb[:].ap[0], [0, NCH]], ti), op=OP.add)
        a = lpool.tile([P, NCH], F32, tag="a2")
        nc.vector.scalar_tensor_tensor(out=a[:], in0=u[:], scalar=cfg.NEG,
                                       in1=u[:], op0=OP.mult, op1=OP.max)
        ex = lpool.tile([P, NCH], BF16, tag="ex2")
        nc.scalar.activation(ex[:], a[:], AF.Exp)
        w4 = lpool.tile([P, NCH, 4], BF16, tag="w4")
        nc.vector.tensor_tensor(
            out=w4[:], in0=mq[:],
            in1=_ap(ex[:], [ex[:].ap[0], [1, NCH], [0, 4]]), op=OP.mult)
        # messages M4[p, (k,q), f] = feat_q[f] * w4[k, q]  (k-major contig)
        M4 = mpool.tile([P, 4 * NCH, OUT], BF16, tag="M4")
        for q in range(4):
            nc.vector.tensor_tensor(
                out=_ap(M4[:], [M4[:].ap[0], [4 * OUT, NCH], [1, OUT]],
                        q * OUT),
                in0=_ap(G2[:], [GP, [4 * QROW, NCH], [1, OUT]], q * QROW),
                in1=_ap(w4[:], [w4[:].ap[0], [4, NCH], [0, OUT]], q),
                op=OP.mult)
        # den: in-place bf16 tree on w4 (after M4 mults consumed it)
        n = 4 * NCH
        while n > 1:
            h_ = n // 2
            nc.vector.tensor_tensor(
                out=_ap(w4[:], [w4[:].ap[0], [1, h_]]),
                in0=_ap(w4[:], [w4[:].ap[0], [1, h_]]),
                in1=_ap(w4[:], [w4[:].ap[0], [1, h_]], n - h_),
                op=OP.add)
            n -= h_
        den = opool.tile([P, 1], F32, tag="den2")
        nc.vector.tensor_scalar_add(
            den[:], _ap(w4[:], [w4[:].ap[0], [1, 1]]), 1e-20)
        n = 4 * NCH
        while n > 4:
            h_ = n // 2
            nc.vector.tensor_tensor(out=M4[:, 0:h_, :], in0=M4[:, 0:h_, :],
                                    in1=M4[:, n - h_:n, :], op=OP.add)
            n -= h_
        agg_a = opool.tile([P, OUT], F32, tag="agg_a2")
        nc.vector.tensor_tensor(out=agg_a[:], in0=M4[:, 0, :],
                                in1=M4[:, 1, :], op=OP.add)
        agg_b = opool.tile([P, OUT], F32, tag="agg_b2")
        if n == 4:
            nc.vector.tensor_tensor(out=agg_b[:], in0=M4[:, 2, :],
                                    in1=M4[:, 3, :], op=OP.add)
        elif n == 3:
            nc.vector.tensor_copy(agg_b[:], M4[:, 2, :])
        else:
            nc.vector.memset(agg_b[:], 0.0)
        agg = opool.tile([P, OUT], F32, tag="agg2")
        nc.vector.tensor_tensor(out=agg[:], in0=agg_a[:], in1=agg_b[:],
                                op=OP.add)

        rcp = opool.tile([P, 1], F32, tag="rcp")
        nc.vector.reciprocal(rcp[:], den[:])
        h2 = opool.tile([P, OUT], F32, tag="h2")
        nc.vector.tensor_tensor(
            out=h2[:], in0=agg[:],
            in1=_ap(rcp[:], [rcp[:].ap[0], [0, OUT]]), op=OP.mult)
        m = opool.tile([P, 1], F32, tag="m")
        nc.vector.tensor_reduce(out=m[:], in_=h2[:],
                                axis=mybir.AxisListType.X, op=OP.max)
        tm = opool.tile([P, OUT], F32, tag="tm")
        nc.vector.tensor_tensor(
            out=tm[:], in0=h2[:],
            in1=_ap(m[:], [m[:].ap[0], [0, OUT]]), op=OP.subtract)
        pex = opool.tile([P, OUT], F32, tag="pex")
        ssum = opool.tile([P, 1], F32, tag="ss")
        nc.scalar.activation(pex[:], tm[:], AF.Exp, accum_out=ssum[:])
        ln = opool.tile([P, 1], F32, tag="ln")
        nc.scalar.activation(ln[:], ssum[:], AF.Ln)
        res = opool.tile([P, OUT], F32, tag="res")
        nc.vector.tensor_tensor(
            out=res[:], in0=tm[:],
            in1=_ap(ln[:], [ln[:].ap[0], [0, OUT]]), op=OP.subtract)
        nc.sync.dma_start(out=t["outp"][ti * P:(ti + 1) * P, :], in_=res[:])
        c0 += NCH


def _decl_a(nc, cfg):
    t = {}
    WCOLS = cfg.F + 2 * cfg.H
    W2COLS = cfg.OUT + 2

    def inp(name, shape, dt):
        t[name] = nc.dram_tensor(name, shape, dt, kind="ExternalInput").ap()

    inp("xT", [P, cfg.N_PAD], BF16)
    inp("wpack", [P, WCOLS], BF16)
    inp("w2pack", [P, W2COLS], BF16)
    inp("ident", [P, P], F32)
    inp("idxA", [P, cfg.NCHSUM * 8], I16)
    inp("pe", [P, cfg.NCHSUM], BF16)
    inp("po", [P, cfg.NCHSUM], BF16)
    t["table1"] = nc.dram_tensor("table1", [cfg.N_PAD, 256], BF16,
                                 kind="Internal").ap()
    t["slab"] = nc.dram_tensor("slab", [cfg.NPC_PAD, W2COLS], F32,
                               kind="ExternalOutput").ap()
    return t


def _decl_b(nc, cfg):
    t = {}

    def inp(name, shape, dt):
        t[name] = nc.dram_tensor(name, shape, dt, kind="ExternalInput").ap()

    inp("table2", [cfg.N_PAD, 32], BF16)
    inp("s2sb", [P, cfg.TPC], F32)
    inp("idxB", [P, cfg.NCHSUM * 8], I16)
    inp("mq4", [P, cfg.NCHSUM * 4], BF16)
    t["outp"] = nc.dram_tensor("outp", [cfg.NPC_PAD, cfg.OUT], F32,
                               kind="ExternalOutput").ap()
    return t


_ACT_PATCHED = False


def _patch_act_tables():
    """Steer the act-table chooser to the set containing exp AND ln AND
    copy, so kernels mixing Exp/Ln/Copy don't thrash ACT_TABLE_LOADs per
    tile. Set IDs are positional (index into act_info.json), so the dict
    ORDER must not change — only membership is edited: exp/ln are removed
    from every other set, making the combined set the only candidate."""
    global _ACT_PATCHED
    if _ACT_PATCHED:
        return
    _ACT_PATCHED = True
    orig = bacc.get_activation_tables
    exp_t = mybir.ActivationFunctionType.Exp
    ln_t = mybir.ActivationFunctionType.Ln
    key = "natural_log_exp_and_others"

    def patched(arch):
        tabs = dict(orig(arch))
        if key not in tabs:
            return tabs
        out = {}
        for name, funcs in tabs.items():
            if name != key:
                funcs = set(funcs) - {exp_t, ln_t}
            out[name] = funcs
        return out

    bacc.get_activation_tables = patched


def _compile(build_fn, decl_fn, cfg):
    _patch_act_tables()
    nc = bacc.Bacc("TRN2", target_bir_lowering=False, debug=False,
                   enable_asserts=False, num_devices=cfg.NCORES,
                   num_swdge_queues=NQ)
    t = decl_fn(nc, cfg)
    with tile.TileContext(nc) as tc:
        build_fn(tc, cfg, t)
    nc.compile()
    return nc


def _host_prep_weights(cfg, W1, att_src1, att_dst1, W2, att_src2, att_dst2):
    A_d1 = _blockdiag_att(np.asarray(att_dst1, np.float32), cfg.H, cfg.HID,
                          cfg.F)
    A_s1 = _blockdiag_att(np.asarray(att_src1, np.float32), cfg.H, cfg.HID,
                          cfg.F)
    W1T = np.asarray(W1, np.float32).T.copy()
    wpack = np.concatenate([W1T, W1T @ A_d1, W1T @ A_s1], axis=1)
    W2T = np.asarray(W2, np.float32).T.copy()
    a_d2 = np.asarray(att_dst2, np.float32).reshape(cfg.OUT, 1)
    a_s2 = np.asarray(att_src2, np.float32).reshape(cfg.OUT, 1)
    w2pack = np.concatenate([W2T, W2T @ a_d2, W2T @ a_s2], axis=1)
    return wpack.astype(BF), w2pack.astype(BF)


_CACHE = {}


def _get_kernels(cfg):
    key = (cfg.N, cfg.E, cfg.NCORES, tuple(cfg.NCH))
    if key not in _CACHE:
        nca = _compile(_build_a, _decl_a, cfg)
        ncb = _compile(_build_b, _decl_b, cfg)
        _CACHE[key] = (nca, ncb)
    return _CACHE[key]


def run(cfg, inputs, runner=None):
    x = np.asarray(inputs["x"], np.float32)
    edge_index = np.asarray(inputs["edge_index"], np.int64)
    pc, pi = _prep_graph(cfg, edge_index)
    wpack, w2pack = _host_prep_weights(
        cfg, inputs["W1"], inputs["att_src1"], inputs["att_dst1"],
        inputs["W2"], inputs["att_src2"], inputs["att_dst2"])

    x_pi = np.zeros((cfg.N_PAD, cfg.IN), np.float32)
    x_pi[pi] = x
    ident = np.eye(P, dtype=np.float32)

    nca, ncb = _get_kernels(cfg)

    if runner is None:
        def runner(nc, in_maps):
            r = bass_utils.run_bass_kernel_spmd(
                nc, in_maps, core_ids=list(range(cfg.NCORES)))
            return r.results

    in_maps_a = []
    for c in range(cfg.NCORES):
        x_rot = np.roll(x_pi, -c * cfg.NPC_PAD, axis=0)
        in_maps_a.append(dict(
            xT=np.ascontiguousarray(x_rot.T).astype(BF), wpack=wpack,
            w2pack=w2pack, ident=ident, idxA=pc["idxA"][c],
            pe=pc["pe"][c], po=pc["po"][c]))
    res_a = runner(nca, in_maps_a)

    # host repack: slab (feat2|d2|s2 per own node, pi order) -> table2
    W2C = cfg.OUT + 2
    slab_pi = np.zeros((cfg.N_PAD, W2C), np.float32)
    for c in range(cfg.NCORES):
        slab_pi[c * cfg.NPC_PAD:(c + 1) * cfg.NPC_PAD] = \
            np.asarray(res_a[c]["slab"], np.float32)
    tab2_pi = np.zeros((cfg.N_PAD, 32), BF)
    tab2_pi[:, 0:cfg.OUT + 1] = slab_pi[:, 0:cfg.OUT + 1].astype(BF)

    in_maps_b = []
    for c in range(cfg.NCORES):
        tab2_rot = np.ascontiguousarray(
            np.roll(tab2_pi, -c * cfg.NPC_PAD, axis=0))
        s2sb = np.ascontiguousarray(
            slab_pi[c * cfg.NPC_PAD:(c + 1) * cfg.NPC_PAD, cfg.OUT + 1]
            .reshape(cfg.TPC, P).T).astype(np.float32)
        in_maps_b.append(dict(
            table2=tab2_rot, s2sb=s2sb, idxB=pc["idxB"][c],
            mq4=pc["mq4"][c]))
    res_b = runner(ncb, in_maps_b)

    out_pi = np.zeros((cfg.N_PAD, cfg.OUT), np.float32)
    for c in range(cfg.NCORES):
        out_pi[c * cfg.NPC_PAD:(c + 1) * cfg.NPC_PAD] = \
            np.asarray(res_b[c]["outp"], np.float32)
    return out_pi[pi]


def kernel(**inputs):
    cfg = Cfg(N=50000, E=1600000, ncores=8)
    return run(cfg, inputs)
